# revision 1
# baseline (speedup 1.0000x reference)
"""Trainium2 Bass kernel for nn_ConvTM2d (Tsetlin-machine conv layer).

Reference computation (jax, fp32):
    patches = unfold(x, 3x3, pad=1)                        # [B, 576, 3136]
    lits    = [patches, 1-patches]                         # [B*L, 1152]
    include = (sigmoid(w_include) > 0.5)                   # binary [256, 1152]
    clauses = exp(log(lits + 1e-6) @ include.T)            # [B*L, 256]
    logits  = clauses @ vote.T                             # [B*L, 128]
    out     = logits -> [B, 128, 56, 56]

Device formulation: the unfold+GEMM is a 3x3 convolution over a 128-channel
"log literal" image (64 ch of log(x+eps), 64 ch of log(1-x+eps)), evaluated
as 9 shifted matmuls accumulating in PSUM, followed by exp (ACT) and a 1x1
conv with vote (2 more matmuls). Zero-padding in x-space becomes constant
borders in log-space (log(eps) / log(1+eps)).

Sharding: data-parallel over batch B=16 -> 2 batches per core x 8 cores.
w_include binarization + transpose into matmul-stationary layout is done
once on host (tiny: ~300K elements).

Schedule notes:
 - input x is DMA'd in 8-row slabs; complement literals come from a
   cross-partition DVE op (write p64-127 from p0-63); Ln runs per slab so
   the first conv matmuls start as soon as rows 0-9 of the padded
   log-literal image exist.
 - all Ln slabs (both batches) are emitted before any Exp so the ACT
   engine loads each activation table exactly once.
 - a burst of dummy matmuls at t~0 keeps the PE busy through the HAM
   activity window so real matmuls run at 2.4 GHz from the start.
"""

import numpy as np
import ml_dtypes

EPS = 1e-6
B_FULL = 16
N_CORES = 8
B_PER_CORE = B_FULL // N_CORES
C_IN = 64
H = W = 56
HP = WP = 58  # padded
N_CLAUSES = 256
N_CLASSES = 128
R_TILE = 8  # output rows per matmul tile -> free dim 8*56=448
N_RTILES = H // R_TILE
N_WARM_MM = 13

LOG_EPS = float(np.log(EPS))            # border, x-literal half
LOG_1PEPS = float(np.log(1.0 + EPS))    # border, complement half

_CACHE = {}

# Test-only: scale log-literals by this factor so clause sums don't underflow
# exp() and the full dataflow can be validated numerically. None in production.
_DEBUG_SCALE = None


def _build_program():
    import concourse.bacc as bacc
    import concourse.mybir as mybir
    import concourse.tile as tile
    from concourse._compat import get_trn_type

    f32 = mybir.dt.float32
    bf16 = mybir.dt.bfloat16
    Ln = mybir.ActivationFunctionType.Ln
    Exp = mybir.ActivationFunctionType.Exp
    mult = mybir.AluOpType.mult
    add = mybir.AluOpType.add

    nc = bacc.Bacc(
        get_trn_type() or "TRN2",
        target_bir_lowering=False,
        debug=False,
        enable_asserts=False,
    )

    x_d = nc.dram_tensor("xs", [B_PER_CORE, C_IN, H, W], f32, kind="ExternalInput")
    w_d = nc.dram_tensor("wstat", [128, 9, N_CLAUSES], bf16, kind="ExternalInput")
    v_d = nc.dram_tensor("voteT", [128, N_CLAUSES], bf16, kind="ExternalInput")
    y_d = nc.dram_tensor("y", [B_PER_CORE, N_CLASSES, H, W], f32, kind="ExternalOutput")

    with tile.TileContext(nc) as tc:
        with tc.tile_pool(name="wpool", bufs=1) as wpool, \
             tc.tile_pool(name="xpool", bufs=2) as xpool, \
             tc.tile_pool(name="lpool", bufs=2) as lpool, \
             tc.tile_pool(name="cpool", bufs=6) as cpool, \
             tc.tile_pool(name="opool", bufs=4) as opool:

            # --- constants / dummy tiles (DVE is idle early) ---
            wzb = wpool.tile([128, R_TILE * W], bf16)
            nc.vector.memset(wzb[:], 0.0)
            eps_t = wpool.tile([128, 1], f32)
            nc.vector.memset(eps_t[:], EPS)

            # Pre-load the Ln ACT table off the critical path with a dummy
            # activation that only depends on the eps memset.
            dact = wpool.tile([128, 1], f32)
            ln_insts, exp_insts = [], []
            ln_insts.append(
                nc.scalar.activation(dact[:], eps_t[:], Ln, bias=eps_t[:]))

            # --- PE HAM warmup: ~3.5us of small dummy matmuls starting right
            # after the engine preamble so the PE is at 2.4GHz when the first
            # real matmul issues. The pool closes afterwards, freeing its
            # PSUM bank for cpsum.
            with tc.tile_pool(name="warmps", bufs=1, space="PSUM") as warmps:
                wps = warmps.tile([64, R_TILE * W], f32)
                for _ in range(N_WARM_MM):
                    nc.tensor.matmul(wps[:], wzb[:, 0:64], wzb[:],
                                     start=True, stop=True)

            # --- weights in (sync queue; x goes on the gpsimd queue) ---
            # wstat lands in two pieces so the first LDWEIGHTS isn't gated on
            # the whole 590KB.
            wsb = wpool.tile([128, 9, N_CLAUSES], bf16)
            nc.sync.dma_start(wsb[:, 0:3, :], w_d[:, 0:3, :])
            nc.sync.dma_start(wsb[:, 3:9, :], w_d[:, 3:9, :])
            vsb = wpool.tile([128, N_CLAUSES], bf16)
            nc.sync.dma_start(vsb[:], v_d[:])

            # --- log-literal images ---
            # x arrives in 2 chunks per batch (fewer, bigger DMAs: the ~600ns
            # per-DMA issue cost on an engine queue adds up). Batch 1's DMAs
            # are held back behind batch 0's second Ln slab so they don't
            # steal HBM bandwidth from the critical-path chunk.
            # x chunk schedule: (batch, lo, hi, index of Ln gating the DMA or
            # None for immediate issue). Later chunks are held behind early Ln
            # slabs so the critical rows-0-15 + wstat transfers get the full
            # HBM bandwidth; each chunk still lands before the slab that
            # needs it.
            x2s, Ls = [], []
            held = []  # (dma_inst, gate_ln_index)
            for b in range(B_PER_CORE):
                x2 = xpool.tile([128, H, W], f32, tag="x2")
                L = lpool.tile([128, HP, WP], bf16, tag="L")
                x2s.append(x2)
                Ls.append(L)
            chunk_plan = [
                (0, 0, 16, None),   # slabs 0-1: critical path
                (0, 16, 56, None),  # rest of b0, queued behind on gpsimd
                (1, 0, 28, 1),      # b1 held until b0 slab-0 Ln ran
                (1, 28, 56, 2),
            ]
            dma_gates = []
            for b, lo, hi, gate in chunk_plan:
                # immediate chunk on the gpsimd queue; held chunks on sync so
                # their semaphore waits don't block the border memsets below
                eng = nc.gpsimd if gate is None else nc.sync
                dma = eng.dma_start(
                    x2s[b][0:64, lo:hi, :], x_d[b, :, lo:hi, :])
                if gate is not None:
                    dma_gates.append((dma, gate))
            for b in range(B_PER_CORE):
                L = Ls[b]
                # borders: log(0 + eps) on x-half, log(1 - 0 + eps) on 1-x half
                nc.gpsimd.memset(L[0:64, 0, :], LOG_EPS)
                nc.gpsimd.memset(L[64:128, 0, :], LOG_1PEPS)
                nc.gpsimd.memset(L[0:64, HP - 1, :], LOG_EPS)
                nc.gpsimd.memset(L[64:128, HP - 1, :], LOG_1PEPS)
                nc.gpsimd.memset(L[0:64, 1:HP - 1, 0], LOG_EPS)
                nc.gpsimd.memset(L[64:128, 1:HP - 1, 0], LOG_1PEPS)
                nc.gpsimd.memset(L[0:64, 1:HP - 1, WP - 1], LOG_EPS)
                nc.gpsimd.memset(L[64:128, 1:HP - 1, WP - 1], LOG_1PEPS)
            for b in range(B_PER_CORE):
                x2, L = x2s[b], Ls[b]
                for s in range(N_RTILES):
                    r0 = s * R_TILE
                    sl = slice(r0, r0 + R_TILE)
                    # complement literals: p64-127 <- 1 - p0-63
                    nc.vector.tensor_scalar(
                        x2[64:128, sl, :], x2[0:64, sl, :], -1.0, 1.0, mult, add)
                    ln_insts.append(nc.scalar.activation(
                        L[:, 1 + r0:1 + r0 + R_TILE, 1:WP - 1], x2[:, sl, :],
                        Ln, bias=eps_t[:]))
                if _DEBUG_SCALE is not None:
                    nc.vector.tensor_scalar_mul(L[:], L[:], float(_DEBUG_SCALE))
            for dma, gate in dma_gates:
                tile.add_dep_helper(dma.ins, ln_insts[gate].ins, sync=True,
                                    reason="stagger x DMA behind critical path")

            # --- conv + exp + vote ---
            with tc.tile_pool(name="cpsum", bufs=7, space="PSUM") as cpsum, \
                 tc.tile_pool(name="lpsum", bufs=1, space="PSUM") as lpsum:
                for b in range(B_PER_CORE):
                    L = Ls[b]
                    for r in range(N_RTILES):
                        r0 = r * R_TILE
                        cls = []
                        for cc in range(2):
                            cps = cpsum.tile([128, R_TILE, W], f32)
                            for ij in range(9):
                                i, j = divmod(ij, 3)
                                nc.tensor.matmul(
                                    cps[:],
                                    wsb[:, ij, cc * 128:(cc + 1) * 128],
                                    L[:, r0 + i:r0 + i + R_TILE, j:j + W],
                                    start=(ij == 0),
                                    stop=(ij == 8),
                                )
                            C = cpool.tile([128, R_TILE, W], bf16)
                            exp_insts.append(
                                nc.scalar.activation(C[:], cps[:], Exp))
                            cls.append(C)
                        lps = lpsum.tile([128, R_TILE, W], f32)
                        nc.tensor.matmul(lps[:], vsb[:, 0:128], cls[0][:],
                                         start=True, stop=False)
                        nc.tensor.matmul(lps[:], vsb[:, 128:256], cls[1][:],
                                         start=False, stop=True)
                        o = opool.tile([128, R_TILE, W], f32)
                        nc.vector.tensor_copy(o[:], lps[:])
                        nc.sync.dma_start(y_d[b, :, r0:r0 + R_TILE, :], o[:])

                # Keep ACT phases contiguous (all Ln, then all Exp): a stray
                # Ln between Exps costs two 1.3us ACT_TABLE_LOADs + PE stall.
                for e in exp_insts:
                    tile.add_dep_helper(e.ins, ln_insts[-1].ins, sync=False,
                                        reason="ACT table phase order")

    nc.compile()
    return nc


def _host_prep(w_include, vote):
    bf16 = ml_dtypes.bfloat16
    include = (w_include > 0.0).astype(np.float32)  # sigmoid(w) > 0.5 <=> w > 0
    incT = np.ascontiguousarray(include.T)          # [1152, 256]
    top = incT[:576].reshape(C_IN, 9, N_CLAUSES)    # x-literal half, [c, ij, m]
    bot = incT[576:].reshape(C_IN, 9, N_CLAUSES)    # complement half
    wstat = np.ascontiguousarray(
        np.concatenate([top, bot], axis=0)).astype(bf16)  # [128, 9, 256]

    voteT = np.ascontiguousarray(vote.T)            # [256, 128] = [clause, class]
    vT = np.ascontiguousarray(
        np.concatenate([voteT[0:128], voteT[128:256]], axis=1)).astype(bf16)
    return wstat, vT


def kernel(x, w_include, vote, _trace=False):
    from concourse import bass_utils

    x = np.ascontiguousarray(np.asarray(x, dtype=np.float32))
    wstat, vT = _host_prep(np.asarray(w_include, dtype=np.float32),
                           np.asarray(vote, dtype=np.float32))

    if "nc" not in _CACHE:
        _CACHE["nc"] = _build_program()
    nc = _CACHE["nc"]

    in_maps = [
        {
            "xs": np.ascontiguousarray(
                x[core * B_PER_CORE:(core + 1) * B_PER_CORE]),
            "wstat": wstat,
            "voteT": vT,
        }
        for core in range(N_CORES)
    ]
    res = bass_utils.run_bass_kernel_spmd(
        nc, in_maps, core_ids=list(range(N_CORES)), trace=_trace,
    )
    out = np.concatenate([r["y"] for r in res.results], axis=0)
    if _trace:
        _CACHE["last_results"] = res
    return out



# revision 14
# speedup vs baseline: 1.4031x; 1.4031x over previous
"""Trainium2 Bass kernel for nn_ConvTM2d (Tsetlin-machine conv layer).

Reference computation (jax, fp32):
    patches = unfold(x, 3x3, pad=1)                        # [B, 576, 3136]
    lits    = [patches, 1-patches]                         # [B*L, 1152]
    include = (sigmoid(w_include) > 0.5)                   # binary [256, 1152]
    clauses = exp(log(lits + 1e-6) @ include.T)            # [B*L, 256]
    logits  = clauses @ vote.T                             # [B*L, 128]
    out     = logits -> [B, 128, 56, 56]

Device formulation: the unfold+GEMM is a 3x3 convolution over a 128-channel
"log literal" image (64 ch of log(x+eps), 64 ch of log(1-x+eps)), evaluated
as shifted matmuls accumulating in PSUM, followed by exp (ACT) and a 1x1
conv with vote. Zero-padding in x-space becomes constant borders in
log-space (log(eps) / log(1+eps)).

fp8 DoubleRow: weights are binary {0,1} (exact in fp8e4) and log-literals
fit fp8e4's range, so all matmuls run in fp8 DoubleRow perf mode, which
contracts 2 K-planes (256 rows) per pass at ~1 PE cycle per output row
(2x the bf16 MAC rate). The 9 conv taps pair into 5 DoubleRow matmuls per
clause half (the odd tap pairs with a zero-weight dummy); the moving
operand is a hand-built 4-dim access pattern [channel, tap-pair-delta,
row, col] over the padded log-literal image. The 256-clause vote
contraction is a single DoubleRow matmul whose K-halves are the two
clause banks.

Sharding: data-parallel over batch B=16 -> 2 batches per core x 8 cores.
w_include binarization + DoubleRow-packing is done once on host (tiny).

Schedule notes (from NTFF traces):
 - the ACT table set `natural_log_exp_and_others` (id 6) is preloaded
   explicitly at t~0: one 1.28us load serves every Ln AND Exp, so Ln and
   Exp interleave freely with no table thrash and no phase barrier.
 - batch 1's Ln slabs are emitted between batch 0's first Exps, so ACT
   fills its x-DMA wait with useful work and batch 1's convs are never
   gated on a late Ln phase.
 - vote matmuls are emitted 2 tiles behind their conv so the in-order PE
   queue fills all 3 conv PSUM buffers instead of stalling on the first
   Exp.
 - y is DMA'd from contiguous [8x56] SBUF tiles, alternating gpsimd/sync
   queues, so there is no output drain tail; x streams in slab-sized
   chunks aligned with the Ln slabs.
 - a burst of dummy matmuls at t~0 keeps the PE busy through the HAM
   activity window so real matmuls run near 2.4 GHz from the start.
"""

import numpy as np
import ml_dtypes

EPS = 1e-6
B_FULL = 16
N_CORES = 8
B_PER_CORE = B_FULL // N_CORES
C_IN = 64
H = W = 56
HP = WP = 58  # padded
FLAT = HP * WP
N_CLAUSES = 256
N_CLASSES = 128
R_TILE = 8  # output rows per tile
N_RTILES = H // R_TILE
N_TILES = B_PER_CORE * N_RTILES
NOUT = R_TILE * W  # 448 real outputs per tile
RUN = (R_TILE - 1) * WP + W  # 462: flat moving-run length per tile
N_WARM_MM = 8
VOTE_LAG = 2  # tiles the exp/vote stage trails the conv stage by
ACT_SET_LN_EXP = 6  # act_info.json id of natural_log_exp_and_others

LOG_EPS = float(np.log(EPS))            # border, x-literal half
LOG_1PEPS = float(np.log(1.0 + EPS))    # border, complement half

# tap ij = 3*i + j; pairs for DoubleRow (A, B), B=None -> zero-weight dummy
TAP_PAIRS = [(0, 1), (2, 3), (4, 5), (6, 7), (8, None)]
# Ln slab row ranges per batch (x-image rows)
LN_SLABS = [(0, 10), (10, 33), (33, 56)]

_CACHE = {}

# Test-only hook kept for test.py compatibility (unused in production).
_DEBUG_SCALE = None


def _tap_off(ij):
    return (ij // 3) * WP + (ij % 3)


def _build_program():
    import concourse.bacc as bacc
    import concourse.mybir as mybir
    import concourse.tile as tile
    from concourse.ap import AP
    from concourse._compat import get_trn_type

    f32 = mybir.dt.float32
    bf16 = mybir.dt.bfloat16
    fp8 = mybir.dt.float8e4
    Ln = mybir.ActivationFunctionType.Ln
    Exp = mybir.ActivationFunctionType.Exp
    DR = mybir.MatmulPerfMode.DoubleRow
    mult = mybir.AluOpType.mult
    add = mybir.AluOpType.add

    nc = bacc.Bacc(
        get_trn_type() or "TRN2",
        target_bir_lowering=False,
        debug=False,
        enable_asserts=False,
    )

    x_d = nc.dram_tensor("xs", [B_PER_CORE, C_IN, H, W], f32, kind="ExternalInput")
    w_d = nc.dram_tensor("wconv", [128, 2, 5, 2, 128], fp8, kind="ExternalInput")
    v_d = nc.dram_tensor("wvote", [128, 2, 128], fp8, kind="ExternalInput")
    y_d = nc.dram_tensor("y", [B_PER_CORE, N_CLASSES, H, W], f32, kind="ExternalOutput")

    with tile.TileContext(nc) as tc:
        with tc.tile_pool(name="wpool", bufs=1) as wpool, \
             tc.tile_pool(name="xpool", bufs=2) as xpool, \
             tc.tile_pool(name="lpool", bufs=2) as lpool, \
             tc.tile_pool(name="cpool", bufs=4) as cpool, \
             tc.tile_pool(name="opool", bufs=4) as opool:

            # --- one ACT table load serves every Ln and Exp in the kernel
            nc.scalar.add_instruction(mybir.InstLoadActFuncSet(
                name=nc.get_next_instruction_name(), ins=[], outs=[],
                act_func_set_id=ACT_SET_LN_EXP))

            eps_t = wpool.tile([128, 1], f32)
            nc.vector.memset(eps_t[:], EPS)
            wzb = wpool.tile([128, NOUT], bf16)
            nc.vector.memset(wzb[:], 0.0)

            # --- PE HAM warmup: dummy matmuls bridge the ramp window so
            # real matmuls run near 2.4GHz.
            with tc.tile_pool(name="warmps", bufs=1, space="PSUM") as warmps:
                wps = warmps.tile([64, NOUT], f32)
                for _ in range(N_WARM_MM):
                    nc.tensor.matmul(wps[:], wzb[:, 0:64], wzb[:],
                                     start=True, stop=True)

            # --- weights in: conv halves on sync, vote on scalar queue ---
            wcb = wpool.tile([128, 2, 5, 2, 128], fp8)
            nc.sync.dma_start(wcb[:, 0], w_d[:, 0])
            nc.sync.dma_start(wcb[:, 1], w_d[:, 1])
            vsb = wpool.tile([128, 2, 128], fp8)
            nc.scalar.dma_start(vsb[:], v_d[:])

            # --- x in: b0 slab-aligned on gpsimd, b1 in halves on sync ---
            x2s, Ls = [], []
            for b in range(B_PER_CORE):
                x2 = xpool.tile([128, H, W], f32, tag="x2")
                L = lpool.tile([128, HP, WP], fp8, tag="L")
                x2s.append(x2)
                Ls.append(L)
            for b, lo, hi in ((0, 0, 10), (0, 10, 33), (0, 33, 56),
                              (1, 0, 28), (1, 28, 56)):
                eng = nc.gpsimd if b == 0 else nc.sync
                eng.dma_start(x2s[b][0:64, lo:hi, :], x_d[b, :, lo:hi, :])
            scale = 1.0 if _DEBUG_SCALE is None else float(_DEBUG_SCALE)
            for b in range(B_PER_CORE):
                L = Ls[b]
                eng = nc.gpsimd
                # borders: log(0 + eps) on x-half, log(1 - 0 + eps) on 1-x half
                eng.memset(L[0:64, 0, :], LOG_EPS * scale)
                eng.memset(L[64:128, 0, :], LOG_1PEPS * scale)
                eng.memset(L[0:64, HP - 1, :], LOG_EPS * scale)
                eng.memset(L[64:128, HP - 1, :], LOG_1PEPS * scale)
                eng.memset(L[0:64, 1:HP - 1, 0], LOG_EPS * scale)
                eng.memset(L[64:128, 1:HP - 1, 0], LOG_1PEPS * scale)
                eng.memset(L[0:64, 1:HP - 1, WP - 1], LOG_EPS * scale)
                eng.memset(L[64:128, 1:HP - 1, WP - 1], LOG_1PEPS * scale)

            def emit_ln(b, slab):
                lo, hi = LN_SLABS[slab]
                sl = slice(lo, hi)
                x2, L = x2s[b], Ls[b]
                # complement literals: p64-127 <- 1 - p0-63
                nc.vector.tensor_scalar(
                    x2[64:128, sl, :], x2[0:64, sl, :], -1.0, 1.0, mult, add)
                nc.scalar.activation(
                    L[:, 1 + lo:1 + hi, 1:WP - 1], x2[:, sl, :],
                    Ln, bias=eps_t[:])
                if _DEBUG_SCALE is not None:
                    nc.vector.tensor_scalar_mul(
                        L[:, 1 + lo:1 + hi, 1:WP - 1],
                        L[:, 1 + lo:1 + hi, 1:WP - 1], scale)

            for slab in range(len(LN_SLABS)):
                emit_ln(0, slab)

            # --- conv + exp + vote, all fp8 DoubleRow; vote lags conv ---
            with tc.tile_pool(name="cpsum", bufs=VOTE_LAG + 1, space="PSUM") as cpsum, \
                 tc.tile_pool(name="lpsum", bufs=2, space="PSUM") as lpsum:
                cps_q = {}
                yq = [nc.gpsimd, nc.sync]

                def emit_conv(t):
                    b, r = divmod(t, N_RTILES)
                    r0 = r * R_TILE
                    L = Ls[b]
                    cps = cpsum.tile([128, 2, 512], f32)
                    for cc in range(2):
                        for k, (ta, tb) in enumerate(TAP_PAIRS):
                            offa = r0 * WP + _tap_off(ta)
                            delta = (_tap_off(tb) - _tap_off(ta)
                                     if tb is not None else -1)
                            mv = AP(L[:, 0, 0:1].tensor, offa,
                                    [[FLAT, 128], [delta, 2], [1, RUN]])
                            nc.tensor.matmul(
                                cps[:, cc, 0:RUN],
                                wcb[:, cc, k],
                                mv,
                                start=(k == 0),
                                stop=(k == len(TAP_PAIRS) - 1),
                                perf_mode=DR,
                            )
                    cps_q[t] = cps

                def emit_tail(t):
                    b, r = divmod(t, N_RTILES)
                    r0 = r * R_TILE
                    cps = cps_q.pop(t)
                    C = cpool.tile([128, 2, RUN], fp8)
                    nc.scalar.activation(C[:], cps[:, :, 0:RUN], Exp)
                    # batch 1's Ln slabs hide between batch 0's first Exps
                    if t < len(LN_SLABS):
                        emit_ln(1, t)
                    lps = lpsum.tile([128, 512], f32)
                    nc.tensor.matmul(lps[:, 0:RUN], vsb[:], C[:],
                                     start=True, stop=True, perf_mode=DR)
                    o = opool.tile([128, RUN], f32)
                    nc.vector.tensor_copy(o[:], lps[:, 0:RUN])
                    # DMA picks the 8x56 real outputs out of the 462-run
                    oy = AP(o[:, 0:1].tensor, 0,
                            [[RUN, 128], [WP, R_TILE], [1, W]])
                    yq[t % len(yq)].dma_start(
                        y_d[b, :, r0:r0 + R_TILE, :], oy)

                for t in range(N_TILES + VOTE_LAG):
                    if t < N_TILES:
                        emit_conv(t)
                    if t >= VOTE_LAG:
                        emit_tail(t - VOTE_LAG)

    nc.compile()
    return nc


def _host_prep(w_include, vote):
    fp8 = ml_dtypes.float8_e4m3
    include = (w_include > 0.0).astype(np.float32)  # sigmoid(w) > 0.5 <=> w > 0
    incT = np.ascontiguousarray(include.T)          # [1152, 256]
    top = incT[:576].reshape(C_IN, 9, N_CLAUSES)    # x-literal half, [c, ij, m]
    bot = incT[576:].reshape(C_IN, 9, N_CLAUSES)    # complement half
    wstat = np.concatenate([top, bot], axis=0)      # [128, 9, 256]

    # DoubleRow conv stationary: [p, cc, pair, t, m]
    wconv = np.zeros((128, 2, 5, 2, 128), dtype=np.float32)
    for cc in range(2):
        for k, (ta, tb) in enumerate(TAP_PAIRS):
            wconv[:, cc, k, 0, :] = wstat[:, ta, cc * 128:(cc + 1) * 128]
            if tb is not None:
                wconv[:, cc, k, 1, :] = wstat[:, tb, cc * 128:(cc + 1) * 128]
    wconv = np.ascontiguousarray(wconv).astype(fp8)

    # DoubleRow vote stationary: wvote[p, t, m] = vote[m, t*128 + p]
    voteT = np.ascontiguousarray(vote.T)            # [256, 128] = [clause, class]
    wvote = np.ascontiguousarray(
        np.stack([voteT[0:128], voteT[128:256]], axis=1)).astype(fp8)
    return wconv, wvote


def kernel(x, w_include, vote, _trace=False):
    from concourse import bass_utils

    x = np.ascontiguousarray(np.asarray(x, dtype=np.float32))
    wconv, wvote = _host_prep(np.asarray(w_include, dtype=np.float32),
                              np.asarray(vote, dtype=np.float32))

    if "nc" not in _CACHE:
        _CACHE["nc"] = _build_program()
    nc = _CACHE["nc"]

    in_maps = [
        {
            "xs": np.ascontiguousarray(
                x[core * B_PER_CORE:(core + 1) * B_PER_CORE]),
            "wconv": wconv,
            "wvote": wvote,
        }
        for core in range(N_CORES)
    ]
    res = bass_utils.run_bass_kernel_spmd(
        nc, in_maps, core_ids=list(range(N_CORES)), trace=_trace,
    )
    out = np.concatenate(
        [np.asarray(r["y"]).astype(np.float32) for r in res.results], axis=0)
    if _trace:
        _CACHE["last_results"] = res
    return out
